# revision 5
# baseline (speedup 1.0000x reference)
"""Trainium2 Bass kernel for the intra-batch point-cloud contrastive loss.

Math (matches the reference exactly):
  feats   = features_in.reshape(C, M).T    (row-major reinterpret), M = B*N
  labels  = labels_in.reshape(-1)
  sel     = bernoulli(key 42, min(750/(count+1),1)[labels])   (host, jax CPU)
  nv      = feats / ||feats||
  dp      = exp(nv @ nv.T / TEMP), diagonal zeroed
  pos_i   = sum_{j sel, same class} dp_ij ; neg over different class
  loss    = mean over selected i of -log(pos/(pos+neg))

Only selected points contribute, so the device works on the compacted point
set (~37% of M).  Columns are sharded over 8 cores; each core receives the
compacted feature matrix *rolled* so its own columns sit first; the diagonal
then lands at a fixed position for every core.

Per core (L = M_pad/8 local columns, nT = M_pad/128 row chunks), all
device data in bf16 (PE runs 4x faster than fp32; DMA halves):
  mm1 (PE):  G_t = nvT[:, chunk t].T @ nvT[:, :L]     [128, L] fp32 psum
             + for diag chunks a [128,128] (-1e9*I) add at the diag window
  exp:       even chunks on ACT (table exp psum->sbuf bf16),
             odd chunks on DVE (Schraudolph: int16(A*sim+B) bitcast bf16),
             diag chunks always on ACT (so -1e9 -> exp -> exact 0)
  mm2 (PE):  S += W_t.T @ dp_t   (W = sel*onehot(label), [4, L] psum accum)
The host gathers the per-core S blocks and finishes the O(n_sel) epilogue.

nv is packed two-half onto 128 partitions ([64, M_pad] -> [128, M_pad/2])
for full DMA bandwidth; the moving rhs (own L columns) is duplicated onto
partitions 64..127 so bottom-half lhsT chunks share the rhs partition base.
"""

import numpy as np

TEMP = 0.07
NUM_CLASSES = 4
N_CORES = 8
P = 128

# Schraudolph exp for bf16 bit patterns: bf16_bits(exp(x)) ~= int16(A*x + B)
# A = 128/(ln2*TEMP) folds in the 1/TEMP scale; B centered for zero mean
# log-error under uniform mantissa distribution.
A_SCH = 128.0 / (0.6931471805599453 * TEMP)
B_SCH = 16256.0 - 7.33

_NEFF_CACHE = {}
_results = [None]


def _compute_sel(labels_flat):
    """Selection mask, bit-exact with the reference (jax threefry, key 42)."""
    import jax
    import jax.numpy as jnp

    cpu = jax.devices("cpu")[0]
    with jax.default_device(cpu):
        lab_j = jnp.asarray(labels_flat)
        counts = jnp.bincount(lab_j, length=NUM_CLASSES)
        keep_p = jnp.minimum(750.0 / (counts.astype(jnp.float32) + 1.0), 1.0)
        p = keep_p[lab_j]
        sel = jax.random.bernoulli(jax.random.key(42), p)
        return np.asarray(sel)


def _build_kernel(M_pad, use_dve):
    import concourse.bass as bass
    import concourse.mybir as mybir
    import concourse.tile as tile

    L = M_pad // N_CORES          # local columns per core
    nT = M_pad // P               # 128-row chunks
    nL = L // P                   # chunks containing this core's diagonal
    H = M_pad // 2                # two-half packing width
    nH = H // P                   # chunks in the top half
    f32 = mybir.dt.float32
    bf16 = mybir.dt.bfloat16
    i16 = mybir.dt.int16

    # packed bf16 layout (cols):
    #   [0, H)            nv two-half: p<64 -> nv[p, j]; p>=64 -> nv[p-64, H+j]
    #   [H, H+L)          rhs dup: partitions 64.. = nv[0:64, 0:L]
    #   [H+L, +nT*4)      W chunks (mm2 lhsT), w[p, 4t+c] = W[128t+p, c]
    #   [.., +128)        eye
    #   [.., +128)        meye = -1e9 * eye
    o_w = H + L
    o_eye = o_w + nT * NUM_CLASSES
    o_meye = o_eye + P
    CW = o_meye + P

    nc = bass.Bass()
    packed_d = nc.dram_tensor("packed", [P, CW], bf16, kind="ExternalInput")
    s_d = nc.dram_tensor("s_out", [NUM_CLASSES, L], f32, kind="ExternalOutput")

    with tile.TileContext(nc) as tc:
        with (
            tc.tile_pool(name="singles", bufs=1) as singles,
            tc.tile_pool(name="dp", bufs=6) as dp_pool,
            tc.tile_pool(name="ps", bufs=7, space="PSUM") as ps_pool,
            tc.tile_pool(name="acc", bufs=1, space="PSUM") as acc_pool,
        ):
            packed = singles.tile([P, CW], bf16)
            # ONE SWDGE (gpsimd) DMA -> one completion semaphore.  Several
            # DMAs would attach more inline sync waits than this walrus
            # build allows per instruction (see _split_multi_waits).
            nc.gpsimd.dma_start(out=packed[:], in_=packed_d[:])
            rhs_top = packed[0:64, 0:L]
            rhs_bot = packed[64:128, H:H + L]
            w_sb = packed[:, o_w:o_eye]
            eye_sb = packed[:, o_eye:o_meye]
            meye_sb = packed[:, o_meye:CW]

            s_ps = acc_pool.tile([NUM_CLASSES, L], f32)
            # software pipeline: defer mm2 by DEPTH chunks so the in-order
            # PE queue never head-of-line blocks on exp(t) finishing.
            DEPTH = 4
            dps = [None] * nT

            def mm2(t):
                nc.tensor.matmul(
                    s_ps[:], w_sb[:, t * NUM_CLASSES:(t + 1) * NUM_CLASSES],
                    dps[t], start=(t == 0), stop=(t == nT - 1),
                )

            for t in range(nT):
                if t < nH:
                    lhsT = packed[0:64, t * P:(t + 1) * P]
                    rhs = rhs_top
                else:
                    lhsT = packed[64:128, (t - nH) * P:(t - nH + 1) * P]
                    rhs = rhs_bot
                ps = ps_pool.tile([P, L], f32)
                nc.tensor.matmul(ps[:], lhsT, rhs, start=True, stop=(t >= nL))
                if t < nL:
                    # adds -1e9 on the diagonal window so exp() maps it to 0
                    nc.tensor.matmul(
                        ps[:, t * P:(t + 1) * P], eye_sb, meye_sb,
                        start=False, stop=True, skip_group_check=True,
                    )
                on_act = (t < nL) or (not use_dve) or (t % 2 == 0)
                if on_act:
                    dp = dp_pool.tile([P, L], bf16)
                    nc.scalar.activation(
                        dp[:], ps[:], mybir.ActivationFunctionType.Exp,
                        scale=float(1.0 / TEMP),
                    )
                    dps[t] = dp[:]
                else:
                    dp = dp_pool.tile([P, L], i16)
                    nc.vector.tensor_scalar(
                        dp[:], ps[:], float(A_SCH), float(B_SCH),
                        mybir.AluOpType.mult, mybir.AluOpType.add,
                    )
                    dps[t] = dp[:].bitcast(bf16)
                if t >= DEPTH:
                    mm2(t - DEPTH)
            for t in range(nT - DEPTH, nT):
                mm2(t)

            s_sb = singles.tile([NUM_CLASSES, L], f32)
            nc.scalar.copy(s_sb[:], s_ps[:])
            nc.gpsimd.dma_start(out=s_d[:], in_=s_sb[:])

    _split_multi_waits(nc)
    return nc


def _split_multi_waits(nc):
    """Walrus in this toolchain accepts only one inline sync-wait per
    instruction.  Tile's kernel-tail drain aggregates one wait per live
    semaphore, so hoist all but the last wait onto same-engine nops."""
    import concourse.mybir as mybir

    for fn in nc.m.functions:
        for blk in fn.blocks:
            insts = list(blk.instructions)
            out = []
            for inst in insts:
                si = inst.sync_info
                waits = list(si.on_wait) if si is not None and si.on_wait else []
                if len(waits) > 1:
                    for w in waits[:-1]:
                        out.append(mybir.InstNoOp(
                            name=nc.get_next_instruction_name(),
                            engine=inst.engine,
                            bass_nofuse=True,
                            sync_info=mybir.SyncInfo(on_wait=[w], on_update=[]),
                        ))
                    si.on_wait = waits[-1:]
                out.append(inst)
            if len(out) != len(insts):
                blk.instructions = out


def _get_kernel(M_pad, use_dve):
    key = (M_pad, use_dve)
    if key not in _NEFF_CACHE:
        _NEFF_CACHE[key] = _build_kernel(M_pad, use_dve)
    return _NEFF_CACHE[key]


def kernel(features_in, labels_in, _trace=False, _results=_results, _use_dve=True):
    import ml_dtypes
    from concourse.bass_utils import run_bass_kernel_spmd

    features_in = np.asarray(features_in, dtype=np.float32)
    B, C, N = features_in.shape
    M = B * N
    labels = np.asarray(labels_in).reshape(-1).astype(np.int64)

    fT = features_in.reshape(C, M)                      # [C, M] reinterpret
    sel = _compute_sel(labels)
    idx = np.nonzero(sel)[0]
    n_sel = int(idx.size)
    n_div = max(n_sel, 1)

    norms = np.sqrt(np.sum(fT * fT, axis=0, dtype=np.float32)).astype(np.float32)
    nvT = (fT / norms).astype(np.float32)

    lab_sel = labels[idx]
    per_core = N_CORES * P
    M_pad = max(((n_sel + per_core - 1) // per_core) * per_core, per_core)
    L = M_pad // N_CORES
    nT = M_pad // P
    H = M_pad // 2

    nvT_pad = np.zeros((C, M_pad), np.float32)
    nvT_pad[:, :n_sel] = nvT[:, idx]
    nv_bf = nvT_pad.astype(ml_dtypes.bfloat16)
    W = np.zeros((M_pad, NUM_CLASSES), ml_dtypes.bfloat16)
    W[np.arange(n_sel), lab_sel] = 1.0

    eye = np.eye(P, dtype=ml_dtypes.bfloat16)
    meye = (-1e9 * np.eye(P)).astype(ml_dtypes.bfloat16)

    o_w = H + L
    CW = o_w + nT * NUM_CLASSES + 2 * P

    in_maps = []
    for k in range(N_CORES):
        nv_k = np.roll(nv_bf, -L * k, axis=1)
        W_k = np.roll(W, -L * k, axis=0)
        w_arr = W_k.reshape(nT, P, NUM_CLASSES).transpose(1, 0, 2).reshape(
            P, nT * NUM_CLASSES
        )
        packed = np.zeros((P, CW), ml_dtypes.bfloat16)
        packed[0:64, 0:H] = nv_k[:, 0:H]
        packed[64:128, 0:H] = nv_k[:, H:M_pad]
        packed[64:128, H:H + L] = nv_k[:, 0:L]       # rhs dup for base-64 lhsT
        packed[:, o_w:o_w + nT * NUM_CLASSES] = w_arr
        packed[:, o_w + nT * NUM_CLASSES:o_w + nT * NUM_CLASSES + P] = eye
        packed[:, o_w + nT * NUM_CLASSES + P:CW] = meye
        in_maps.append({"packed": packed})

    nc = _get_kernel(M_pad, _use_dve)
    res = run_bass_kernel_spmd(nc, in_maps, core_ids=list(range(N_CORES)),
                               trace=_trace)
    _results[0] = res

    S = np.concatenate([res.results[k]["s_out"] for k in range(N_CORES)], axis=1)
    S = S[:, :n_sel]
    denom = np.sum(S, axis=0, dtype=np.float32).astype(np.float32)
    numer = S[lab_sel, np.arange(n_sel)]
    per = (-np.log(numer / denom)).astype(np.float32)
    loss = np.float32(per.sum(dtype=np.float32) / np.float32(n_div))
    return np.asarray(loss, dtype=np.float32)


# revision 9
# speedup vs baseline: 1.0681x; 1.0681x over previous
"""Trainium2 Bass kernel for the intra-batch point-cloud contrastive loss.

Math (matches the reference exactly):
  feats   = features_in.reshape(C, M).T    (row-major reinterpret), M = B*N
  labels  = labels_in.reshape(-1)
  sel     = bernoulli(key 42, min(750/(count+1),1)[labels])   (host, jax CPU)
  nv      = feats / ||feats||
  dp      = exp(nv @ nv.T / TEMP), diagonal zeroed
  pos_i   = sum_{j sel, same class} dp_ij ; neg over different class
  loss    = mean over selected i of -log(pos/(pos+neg))

Only selected points contribute, so the device works on the compacted point
set (~37% of M).  Columns are sharded over 8 cores; each core receives the
compacted feature matrix *rolled* so its own columns sit first; the diagonal
then lands at a fixed position for every core.

Per core (L = M_pad/8 local columns, nT = M_pad/128 row chunks), all
device data in bf16 (PE runs 4x faster than fp32; DMA halves):
  mm1 (PE):  G_t = nvT[:, chunk t].T @ nvT[:, :L]     [128, L] fp32 psum
             + for diag chunks a [128,128] (-1e9*I) add at the diag window
  exp:       even chunks on ACT (table exp psum->sbuf bf16),
             odd chunks on DVE (Schraudolph: int16(A*sim+B) bitcast bf16),
             diag chunks always on ACT (so -1e9 -> exp -> exact 0)
  mm2 (PE):  S += W_t.T @ dp_t   (W = sel*onehot(label), [4, L] psum accum)
The host gathers the per-core S blocks and finishes the O(n_sel) epilogue.

nv is packed two-half onto 128 partitions ([64, M_pad] -> [128, M_pad/2])
for full DMA bandwidth; the moving rhs (own L columns) is duplicated onto
partitions 64..127 so bottom-half lhsT chunks share the rhs partition base.
"""

import numpy as np

TEMP = 0.07
NUM_CLASSES = 4
N_CORES = 8
P = 128

# Schraudolph exp for bf16 bit patterns: bf16_bits(exp(x)) ~= int16(A*x + B)
# A = 128/(ln2*TEMP) folds in the 1/TEMP scale; B centered for zero mean
# log-error under uniform mantissa distribution.
A_SCH = 128.0 / (0.6931471805599453 * TEMP)
B_SCH = 16256.0 - 7.33

_NEFF_CACHE = {}
_results = [None]


def _compute_sel(labels_flat):
    """Selection mask, bit-exact with the reference (jax threefry, key 42)."""
    import jax
    import jax.numpy as jnp

    cpu = jax.devices("cpu")[0]
    with jax.default_device(cpu):
        lab_j = jnp.asarray(labels_flat)
        counts = jnp.bincount(lab_j, length=NUM_CLASSES)
        keep_p = jnp.minimum(750.0 / (counts.astype(jnp.float32) + 1.0), 1.0)
        p = keep_p[lab_j]
        sel = jax.random.bernoulli(jax.random.key(42), p)
        return np.asarray(sel)


def _build_kernel(M_pad, use_dve, n_warm=6):
    import concourse.bass as bass
    import concourse.mybir as mybir
    import concourse.tile as tile

    L = M_pad // N_CORES          # local columns per core
    nT = M_pad // P               # 128-row chunks
    nL = L // P                   # chunks containing this core's diagonal
    H = M_pad // 2                # two-half packing width
    nH = H // P                   # chunks in the top half
    f32 = mybir.dt.float32
    bf16 = mybir.dt.bfloat16
    i16 = mybir.dt.int16

    # packed bf16 layout (cols):
    #   [0, H)            nv two-half: p<64 -> nv[p, j]; p>=64 -> nv[p-64, H+j]
    #   [H, H+L)          rhs dup: partitions 64.. = nv[0:64, 0:L]
    #   [H+L, +nT*4)      W chunks (mm2 lhsT), w[p, 4t+c] = W[128t+p, c]
    #   [.., +128)        eye
    #   [.., +128)        meye = -1e9 * eye
    o_w = H + L
    o_eye = o_w + nT * NUM_CLASSES
    o_meye = o_eye + P
    CW = o_meye + P

    nc = bass.Bass()
    packed_d = nc.dram_tensor("packed", [P, CW], bf16, kind="ExternalInput")
    s_d = nc.dram_tensor("s_out", [NUM_CLASSES, L], f32, kind="ExternalOutput")

    with tile.TileContext(nc) as tc:
        with (
            tc.tile_pool(name="singles", bufs=1) as singles,
            tc.tile_pool(name="dp", bufs=6) as dp_pool,
            tc.tile_pool(name="ps", bufs=6, space="PSUM") as ps_pool,
            tc.tile_pool(name="acc", bufs=1, space="PSUM") as acc_pool,
            tc.tile_pool(name="wps", bufs=1, space="PSUM") as wps_pool,
        ):
            packed = singles.tile([P, CW], bf16)
            # ONE SWDGE (gpsimd) DMA -> one completion semaphore.  Several
            # DMAs would attach more inline sync waits than this walrus
            # build allows per instruction (see _split_multi_waits).
            nc.gpsimd.dma_start(out=packed[:], in_=packed_d[:])

            # PE p-state warmup: the tensor engine only reaches max clock
            # after ~3us of continuous busy.  It would otherwise sit idle
            # during the input DMA and run the whole kernel at the mid
            # p-state; burn junk matmuls (zeros) during the DMA so the real
            # matmuls start on a hot PE.
            if n_warm:
                warm = singles.tile([P, 512], bf16)
                nc.vector.memset(warm[:], 0.0)
                wps = wps_pool.tile([P, 512], f32)
                for _ in range(n_warm):
                    nc.tensor.matmul(wps[:], warm[:, 0:P], warm[:],
                                     start=True, stop=True)
            rhs_top = packed[0:64, 0:L]
            rhs_bot = packed[64:128, H:H + L]
            w_sb = packed[:, o_w:o_eye]
            eye_sb = packed[:, o_eye:o_meye]
            meye_sb = packed[:, o_meye:CW]

            s_ps = acc_pool.tile([NUM_CLASSES, L], f32)
            # software pipeline: defer mm2 by DEPTH chunks so the in-order
            # PE queue never head-of-line blocks on exp(t) finishing.
            DEPTH = 4
            dps = [None] * nT

            def mm2(t):
                nc.tensor.matmul(
                    s_ps[:], w_sb[:, t * NUM_CLASSES:(t + 1) * NUM_CLASSES],
                    dps[t], start=(t == 0), stop=(t == nT - 1),
                )

            for t in range(nT):
                if t < nH:
                    lhsT = packed[0:64, t * P:(t + 1) * P]
                    rhs = rhs_top
                else:
                    lhsT = packed[64:128, (t - nH) * P:(t - nH + 1) * P]
                    rhs = rhs_bot
                ps = ps_pool.tile([P, L], f32)
                nc.tensor.matmul(ps[:], lhsT, rhs, start=True, stop=(t >= nL))
                if t < nL:
                    # adds -1e9 on the diagonal window so exp() maps it to 0
                    nc.tensor.matmul(
                        ps[:, t * P:(t + 1) * P], eye_sb, meye_sb,
                        start=False, stop=True, skip_group_check=True,
                    )
                on_act = (t < nL) or (not use_dve) or (t % 2 == 0)
                if on_act:
                    dp = dp_pool.tile([P, L], bf16)
                    nc.scalar.activation(
                        dp[:], ps[:], mybir.ActivationFunctionType.Exp,
                        scale=float(1.0 / TEMP),
                    )
                    dps[t] = dp[:]
                else:
                    dp = dp_pool.tile([P, L], i16)
                    nc.vector.tensor_scalar(
                        dp[:], ps[:], float(A_SCH), float(B_SCH),
                        mybir.AluOpType.mult, mybir.AluOpType.add,
                    )
                    dps[t] = dp[:].bitcast(bf16)
                if t >= DEPTH:
                    mm2(t - DEPTH)
            for t in range(nT - DEPTH, nT):
                mm2(t)

            s_sb = singles.tile([NUM_CLASSES, L], f32)
            nc.scalar.copy(s_sb[:], s_ps[:])
            nc.gpsimd.dma_start(out=s_d[:], in_=s_sb[:])

    _split_multi_waits(nc)
    return nc


def _split_multi_waits(nc):
    """Walrus in this toolchain accepts only one inline sync-wait per
    instruction.  Tile's kernel-tail drain aggregates one wait per live
    semaphore, so hoist all but the last wait onto same-engine nops."""
    import concourse.mybir as mybir

    for fn in nc.m.functions:
        for blk in fn.blocks:
            insts = list(blk.instructions)
            out = []
            for inst in insts:
                si = inst.sync_info
                waits = list(si.on_wait) if si is not None and si.on_wait else []
                if len(waits) > 1:
                    for w in waits[:-1]:
                        out.append(mybir.InstNoOp(
                            name=nc.get_next_instruction_name(),
                            engine=inst.engine,
                            bass_nofuse=True,
                            sync_info=mybir.SyncInfo(on_wait=[w], on_update=[]),
                        ))
                    si.on_wait = waits[-1:]
                out.append(inst)
            if len(out) != len(insts):
                blk.instructions = out


def _get_kernel(M_pad, use_dve):
    key = (M_pad, use_dve)
    if key not in _NEFF_CACHE:
        _NEFF_CACHE[key] = _build_kernel(M_pad, use_dve)
    return _NEFF_CACHE[key]


def _sim_ns(M_pad=3072, use_dve=True, n_warm=9):
    """Local cost-model timing (no HW)."""
    from concourse.timeline_sim import TimelineSim
    return TimelineSim(_build_kernel(M_pad, use_dve, n_warm)).simulate()


def kernel(features_in, labels_in, _trace=False, _results=_results, _use_dve=True):
    import ml_dtypes
    from concourse.bass_utils import run_bass_kernel_spmd

    features_in = np.asarray(features_in, dtype=np.float32)
    B, C, N = features_in.shape
    M = B * N
    labels = np.asarray(labels_in).reshape(-1).astype(np.int64)

    fT = features_in.reshape(C, M)                      # [C, M] reinterpret
    sel = _compute_sel(labels)
    idx = np.nonzero(sel)[0]
    n_sel = int(idx.size)
    n_div = max(n_sel, 1)

    norms = np.sqrt(np.sum(fT * fT, axis=0, dtype=np.float32)).astype(np.float32)
    nvT = (fT / norms).astype(np.float32)

    lab_sel = labels[idx]
    per_core = N_CORES * P
    M_pad = max(((n_sel + per_core - 1) // per_core) * per_core, per_core)
    L = M_pad // N_CORES
    nT = M_pad // P
    H = M_pad // 2

    nvT_pad = np.zeros((C, M_pad), np.float32)
    nvT_pad[:, :n_sel] = nvT[:, idx]
    nv_bf = nvT_pad.astype(ml_dtypes.bfloat16)
    W = np.zeros((M_pad, NUM_CLASSES), ml_dtypes.bfloat16)
    W[np.arange(n_sel), lab_sel] = 1.0

    eye = np.eye(P, dtype=ml_dtypes.bfloat16)
    meye = (-1e9 * np.eye(P)).astype(ml_dtypes.bfloat16)

    o_w = H + L
    CW = o_w + nT * NUM_CLASSES + 2 * P

    in_maps = []
    for k in range(N_CORES):
        nv_k = np.roll(nv_bf, -L * k, axis=1)
        W_k = np.roll(W, -L * k, axis=0)
        w_arr = W_k.reshape(nT, P, NUM_CLASSES).transpose(1, 0, 2).reshape(
            P, nT * NUM_CLASSES
        )
        packed = np.zeros((P, CW), ml_dtypes.bfloat16)
        packed[0:64, 0:H] = nv_k[:, 0:H]
        packed[64:128, 0:H] = nv_k[:, H:M_pad]
        packed[64:128, H:H + L] = nv_k[:, 0:L]       # rhs dup for base-64 lhsT
        packed[:, o_w:o_w + nT * NUM_CLASSES] = w_arr
        packed[:, o_w + nT * NUM_CLASSES:o_w + nT * NUM_CLASSES + P] = eye
        packed[:, o_w + nT * NUM_CLASSES + P:CW] = meye
        in_maps.append({"packed": packed})

    nc = _get_kernel(M_pad, _use_dve)
    res = run_bass_kernel_spmd(nc, in_maps, core_ids=list(range(N_CORES)),
                               trace=_trace)
    _results[0] = res

    S = np.concatenate([res.results[k]["s_out"] for k in range(N_CORES)], axis=1)
    S = S[:, :n_sel]
    denom = np.sum(S, axis=0, dtype=np.float32).astype(np.float32)
    numer = S[lab_sel, np.arange(n_sel)]
    per = (-np.log(numer / denom)).astype(np.float32)
    loss = np.float32(per.sum(dtype=np.float32) / np.float32(n_div))
    return np.asarray(loss, dtype=np.float32)
